# revision 12
# baseline (speedup 1.0000x reference)
"""DoRA adapter forward kernel for 8 trn2 NeuronCores.

Math:  dora = dora_B @ dora_A                       [OUT, IN]
       num  = weight + ALPHA * dora                 [OUT, IN]
       s    = m / sqrt(colsum_over_out(num^2))      [1, IN]
       out  = x @ (num * s)^T + bias                (scale folded per-IN column)

Sharding (4x2 grid): 4-way split of the 8192 x-rows, 2-way split of OUT.

Per-core plan (all bulk DMA is contiguous HWDGE on the nc.sync ring; no
DMA-transpose, no SWDGE cast round-trips):

  phase 1: stream W [128 x IC] f32 tiles once.  For each 128x128 sub-tile,
           PE-transpose it into PSUM and accumulate the rank-16 dora^T
           (lhsT = ALPHA*A chunk, rhs = B^T chunk) into the same PSUM
           region, so PSUM holds num^T = W^T + ALPHA*(BA)^T directly.
           One DVE copy per [128, 512] PSUM block materializes a resident
           SBUF num^T (bf16, 16.8 MB).  Column norms come from a fused
           multiply-reduce over the finished num^T tiles (partition-major
           [128, IN/128] layout - no strided DRAM gathers).
  s:       16 KB partial colsum AllReduced across all 8 cores (each o-half
           contributed by MG cores -> reduce = MG * full; folded into the
           sqrt scale), then s = m / sqrt(.).
  phase 2: stream x [128 x IN] f32 row-tiles; PE-transpose each 128x128
           sub-tile; the mandatory PSUM->SBUF copy applies the per-i scale
           s (tensor_scalar_mul) and casts to bf16.  Dense bf16 GEMM with
           xT sub-tiles stationary (reused across 4 N=512 chunks), num^T
           moving, fp32 PSUM accumulation; bias (pre-replicated via a K=1
           ones-matmul) added on the PSUM drain; contiguous row stores.
"""

import sys

if "/opt/trn_rl_repo" not in sys.path:
    sys.path.insert(0, "/opt/trn_rl_repo")

import numpy as np

import concourse.bass as bass
import concourse.mybir as mybir
import concourse.tile as tile
from concourse import bacc
from concourse.bass_utils import run_bass_kernel_spmd
from concourse.masks import make_identity
from concourse.tile_rust import add_dep_helper

F32 = mybir.dt.float32
BF16 = mybir.dt.bfloat16

ALPHA = 16.0
N_CORES = 8
MG, OG = 4, 2  # core grid: 4 m-groups x 2 o-halves

# full problem sizes (hardcoded per contest contract)
B_, S_, IN_FULL, OUT_FULL, R_ = 4, 2048, 4096, 4096, 16
M_FULL = B_ * S_
M_C = M_FULL // MG      # 2048 x-rows per core
O_C = OUT_FULL // OG    # 2048 out-cols per core

FUSE_TDORA = False  # accumulate dora^T into the W^T transpose PSUM group


def build_kernel(M_C=M_C, IN=IN_FULL, O_C=O_C, R=R_, reps=1, n_cores=N_CORES):
    """Build the (core-agnostic SPMD) bass program."""
    IC = min(1024, IN)        # phase-1 W-tile width (i)
    OB = min(512, O_C)        # phase-1 psum block width (o)
    NQ = min(512, O_C)        # phase-2 matmul free width (o)
    n_it = IN // 128
    assert IN % IC == 0 and O_C % OB == 0 and O_C % NQ == 0
    assert M_C % 128 == 0 and IC % 128 == 0 and OB % 128 == 0

    nc = bacc.Bacc("TRN2", target_bir_lowering=False, debug=False,
                   num_devices=n_cores)

    x_in = nc.dram_tensor("x_slice", [M_C, IN], F32, kind="ExternalInput")
    w_own = nc.dram_tensor("w_own", [O_C, IN], F32, kind="ExternalInput")
    bias_in = nc.dram_tensor("bias_own", [1, O_C], F32, kind="ExternalInput")
    m_in = nc.dram_tensor("m_row", [1, IN], F32, kind="ExternalInput")
    a_in = nc.dram_tensor("dora_a", [R, IN], F32, kind="ExternalInput")
    b_own = nc.dram_tensor("dora_b_own", [O_C, R], F32, kind="ExternalInput")
    out_t = nc.dram_tensor("out_slice", [M_C, O_C], F32, kind="ExternalOutput")

    s_drams, cc_outs = [], []
    for rep in range(reps):
        s_drams.append(nc.dram_tensor(f"s_dram{rep}", [128, n_it], F32))
        cc_outs.append(nc.dram_tensor(f"cc_out{rep}", [128, n_it], F32,
                                      addr_space="Shared"))

    with tile.TileContext(nc) as tc:
        for rep in range(reps):
            _emit_rep(tc, nc, rep, M_C, IN, O_C, R, IC, OB, NQ, n_it,
                      x_in, w_own, bias_in, m_in, a_in, b_own, out_t,
                      s_drams[rep], cc_outs[rep])
    nc.compile()
    return nc


def _emit_rep(tc, nc, rep, M_C, IN, O_C, R, IC, OB, NQ, n_it,
              x_in, w_own, bias_in, m_in, a_in, b_own, out_t, s_dram, cc_out):
    ADD, MUL = mybir.AluOpType.add, mybir.AluOpType.mult
    n_ic = IN // IC
    n_ot = O_C // 128
    n_og = O_C // OB          # phase-1 o-groups (OB-wide)
    ob_t = OB // 128          # W row-tiles per o-group
    n_q = O_C // NQ

    with tc.tile_pool(name=f"c{rep}", bufs=1) as const, \
         tc.tile_pool(name=f"nt{rep}", bufs=1) as ntp:
        ident = const.tile([128, 128], F32, tag="ident")
        make_identity(nc, ident[:])
        bias_rep = const.tile([128, O_C], F32, tag="bias_rep")
        s_t = const.tile([128, n_it], F32, tag="s_t")
        nt_tiles = [ntp.tile([128, O_C], BF16, tag=f"nt{it}", name=f"nt{it}")
                    for it in range(n_it)]

        with tc.tile_pool(name=f"su{rep}", bufs=1) as setup, \
             tc.tile_pool(name=f"sups{rep}", bufs=2, space="PSUM") as sups:
            # ---- setup: A (pre-scaled), B^T, bias replicate, m ----
            a_bf = setup.tile([R, IN], BF16, tag="a_bf")
            bt_bf = setup.tile([R, O_C], BF16, tag="bt_bf")
            with tc.tile_pool(name=f"tmp{rep}", bufs=1) as tmp, \
                 tc.tile_pool(name=f"bt{rep}", bufs=2) as btp:
                ones_row = tmp.tile([1, 128], F32, tag="ones_row")
                nc.gpsimd.memset(ones_row[:], 1.0)

                a_raw = tmp.tile([R, IN], F32, tag="a_raw")
                nc.sync.dma_start(out=a_raw[:], in_=a_in[:, :])
                nc.vector.tensor_scalar_mul(a_bf[:], a_raw[:], ALPHA)

                for ot in range(n_ot):
                    b_t = btp.tile([128, R], F32, tag="b_t")
                    nc.sync.dma_start(out=b_t[:],
                                      in_=b_own[ot * 128:(ot + 1) * 128, :])
                    ps = sups.tile([R, 128], F32, tag="bt_ps")
                    nc.tensor.transpose(ps[:], b_t[:], ident[:])
                    nc.vector.tensor_copy(
                        out=bt_bf[:, ot * 128:(ot + 1) * 128], in_=ps[:])

                bias_sb = tmp.tile([1, O_C], F32, tag="bias_sb")
                nc.sync.dma_start(out=bias_sb[0:1, :], in_=bias_in[:, :])
                for oc in range(O_C // 512):
                    ps_b = sups.tile([128, 512], F32, tag="ps_b")
                    nc.tensor.matmul(ps_b[:], lhsT=ones_row[:],
                                     rhs=bias_sb[0:1, oc * 512:(oc + 1) * 512],
                                     start=True, stop=True)
                    nc.vector.tensor_copy(
                        out=bias_rep[:, oc * 512:(oc + 1) * 512], in_=ps_b[:])

            m_t = setup.tile([128, n_it], F32, tag="m_t")
            nc.sync.dma_start(
                out=m_t[:], in_=m_in.ap().rearrange("a (c p) -> (a p) c", p=128))

            # ---- phase 1: build resident num^T; fused colsum ----
            s_col = setup.tile([128, n_it], F32, tag="s_col")
            prod = setup.tile([128, O_C], BF16, tag="prod")
            with tc.tile_pool(name=f"w{rep}", bufs=ob_t + 2) as wp, \
                 tc.tile_pool(name=f"dt{rep}", bufs=3) as dtp, \
                 tc.tile_pool(name=f"p1ps{rep}", bufs=4, space="PSUM") as p1ps:
                for ic in range(n_ic):
                    for og in range(n_og):
                        w_ts = []
                        for j in range(ob_t):
                            ot = og * ob_t + j
                            w_t = wp.tile([128, IC], F32, tag="w_t")
                            nc.sync.dma_start(
                                out=w_t[:],
                                in_=w_own[ot * 128:(ot + 1) * 128,
                                          ic * IC:(ic + 1) * IC])
                            w_ts.append(w_t)
                        for s8 in range(IC // 128):
                            it = ic * (IC // 128) + s8
                            ps = p1ps.tile([128, OB], F32, tag="p1")
                            if FUSE_TDORA:
                                # dora^T first (start zeroes the whole bank),
                                # then W^T sub-transposes accumulate into it
                                nc.tensor.matmul(
                                    ps[:, :],
                                    lhsT=a_bf[:, it * 128:(it + 1) * 128],
                                    rhs=bt_bf[:, og * OB:(og + 1) * OB],
                                    start=True, stop=False)
                                for j in range(ob_t):
                                    nc.tensor.matmul(
                                        ps[:, j * 128:(j + 1) * 128],
                                        lhsT=w_ts[j][:, s8 * 128:(s8 + 1) * 128],
                                        rhs=ident[:], is_transpose=True,
                                        start=False, stop=(j == ob_t - 1))
                                nc.vector.tensor_copy(
                                    out=nt_tiles[it][:, og * OB:(og + 1) * OB],
                                    in_=ps[:])
                            else:
                                # TensorTensor may read only ONE input from
                                # PSUM: round-trip dora^T through SBUF.
                                for j in range(ob_t):
                                    nc.tensor.matmul(
                                        ps[:, j * 128:(j + 1) * 128],
                                        lhsT=w_ts[j][:, s8 * 128:(s8 + 1) * 128],
                                        rhs=ident[:], is_transpose=True,
                                        start=True, stop=True)
                                ps_d = p1ps.tile([128, OB], F32, tag="p1")
                                nc.tensor.matmul(
                                    ps_d[:, :],
                                    lhsT=a_bf[:, it * 128:(it + 1) * 128],
                                    rhs=bt_bf[:, og * OB:(og + 1) * OB],
                                    start=True, stop=True)
                                dt_sb = dtp.tile([128, OB], BF16, tag="dt_sb")
                                nc.vector.tensor_copy(out=dt_sb[:], in_=ps_d[:])
                                nc.vector.tensor_add(
                                    out=nt_tiles[it][:, og * OB:(og + 1) * OB],
                                    in0=ps[:], in1=dt_sb[:])
                    # colsum(num^2) partial for this ic's finished nt tiles:
                    # ACT square with accumulate (the softmax sum pattern)
                    for s8 in range(IC // 128):
                        it = ic * (IC // 128) + s8
                        nc.scalar.activation(
                            prod[:], nt_tiles[it][:],
                            mybir.ActivationFunctionType.Square,
                            0.0, 1.0, 0.0,
                            accum_out=s_col[:, it:it + 1])

            # ---- s = m / sqrt(allreduce(colsum) / MG) ----
            # store on gpsimd so the SWDGE store and the collective share a
            # queue (baseline-proven ordering; no explicit dep onto cc)
            nc.gpsimd.dma_start(out=s_dram[:, :], in_=s_col[:])
            cc = nc.gpsimd.collective_compute(
                "AllReduce", ADD,
                ins=[s_dram.ap()], outs=[cc_out.ap()],
                replica_groups=[list(range(N_CORES))])
            s_raw = setup.tile([128, n_it], F32, tag="s_raw")
            ld = nc.sync.dma_start(out=s_raw[:], in_=cc_out[:, :])
            add_dep_helper(ld.ins, cc.ins, reason="s_raw RAW on collective out")
            s_sq = setup.tile([128, n_it], F32, tag="s_sq")
            nc.scalar.activation(s_sq[:], s_raw[:],
                                 mybir.ActivationFunctionType.Sqrt,
                                 0.0, 1.0 / MG)
            s_rc = setup.tile([128, n_it], F32, tag="s_rc")
            nc.vector.reciprocal(s_rc[:], s_sq[:])
            nc.vector.tensor_mul(out=s_t[:], in0=s_rc[:], in1=m_t[:])

        # ---- phase 2: out = (x^T * s)^T @ num^T + bias ----
        XH = min(2048, IN)  # x half-row width
        n_xh = IN // XH
        with tc.tile_pool(name=f"x{rep}", bufs=3) as xp, \
             tc.tile_pool(name=f"xs{rep}", bufs=n_it + 2) as xsp, \
             tc.tile_pool(name=f"ob{rep}", bufs=2) as obp, \
             tc.tile_pool(name=f"p2t{rep}", bufs=3, space="PSUM") as p2t, \
             tc.tile_pool(name=f"p2o{rep}", bufs=n_q + 1, space="PSUM") as p2o:
            for mt in range(M_C // 128):
                xs_tiles = []
                for xh in range(n_xh):
                    x_t = xp.tile([128, XH], F32, tag="x_t")
                    nc.sync.dma_start(
                        out=x_t[:],
                        in_=x_in[mt * 128:(mt + 1) * 128,
                                 xh * XH:(xh + 1) * XH])
                    for s8 in range(XH // 128):
                        it = xh * (XH // 128) + s8
                        ps_x = p2t.tile([128, 128], F32, tag="ps_x")
                        nc.tensor.transpose(
                            ps_x[:], x_t[:, s8 * 128:(s8 + 1) * 128], ident[:])
                        xs = xsp.tile([128, 128], BF16, tag="xs")
                        nc.vector.tensor_scalar_mul(xs[:], ps_x[:],
                                                    s_t[:, it:it + 1])
                        xs_tiles.append(xs)
                ps_q = [p2o.tile([128, NQ], F32, tag="ps_q", name="ps_q")
                        for _ in range(n_q)]
                for it in range(n_it):
                    for q in range(n_q):
                        nc.tensor.matmul(
                            ps_q[q][:, :],
                            lhsT=xs_tiles[it][:],
                            rhs=nt_tiles[it][:, q * NQ:(q + 1) * NQ],
                            start=(it == 0), stop=(it == n_it - 1))
                o_sb = obp.tile([128, O_C], F32, tag="o_sb")
                for q in range(n_q):
                    nc.vector.tensor_add(
                        out=o_sb[:, q * NQ:(q + 1) * NQ], in0=ps_q[q][:],
                        in1=bias_rep[:, q * NQ:(q + 1) * NQ])
                nc.sync.dma_start(
                    out=out_t[mt * 128:(mt + 1) * 128, :], in_=o_sb[:])


_NC_CACHE = {}


def get_nc(reps=1):
    key = reps
    if key not in _NC_CACHE:
        _NC_CACHE[key] = build_kernel(reps=reps)
    return _NC_CACHE[key]


def make_in_maps(x, weight, bias, m, dora_A, dora_B):
    x = np.ascontiguousarray(np.asarray(x, dtype=np.float32))
    weight = np.ascontiguousarray(np.asarray(weight, dtype=np.float32))
    bias = np.ascontiguousarray(np.asarray(bias, dtype=np.float32))
    m = np.ascontiguousarray(np.asarray(m, dtype=np.float32))
    dora_A = np.ascontiguousarray(np.asarray(dora_A, dtype=np.float32))
    dora_B = np.ascontiguousarray(np.asarray(dora_B, dtype=np.float32))
    xf = x.reshape(M_FULL, IN_FULL)
    in_maps = []
    for c in range(N_CORES):
        g, h = divmod(c, OG)
        o0 = h * O_C
        im = {
            "x_slice": np.ascontiguousarray(xf[g * M_C:(g + 1) * M_C]),
            "w_own": np.ascontiguousarray(weight[o0:o0 + O_C]),
            "bias_own": np.ascontiguousarray(bias[o0:o0 + O_C].reshape(1, O_C)),
            "m_row": np.ascontiguousarray(m.reshape(1, IN_FULL)),
            "dora_a": dora_A,
            "dora_b_own": np.ascontiguousarray(dora_B[o0:o0 + O_C]),
        }
        in_maps.append(im)
    return in_maps


def kernel(x, weight, bias, m, dora_A, dora_B, _trace=False, _trace_kwargs=None):
    in_maps = make_in_maps(x, weight, bias, m, dora_A, dora_B)
    res = run_bass_kernel_spmd(
        get_nc(), in_maps, core_ids=list(range(N_CORES)),
        trace=_trace, **(_trace_kwargs or {}))
    out = np.empty((M_FULL, OUT_FULL), np.float32)
    for c in range(N_CORES):
        g, h = divmod(c, OG)
        out[g * M_C:(g + 1) * M_C, h * O_C:(h + 1) * O_C] = \
            res.results[c]["out_slice"]
    ret = out.reshape(B_, S_, OUT_FULL)
    if _trace:
        return ret, res
    return ret


# revision 15
# speedup vs baseline: 25191.9044x; 25191.9044x over previous
"""DoRA adapter forward kernel for 8 trn2 NeuronCores.

Math:  dora = dora_B @ dora_A                       [OUT, IN]
       num  = weight + ALPHA * dora                 [OUT, IN]
       s    = m / sqrt(colsum_over_out(num^2))      [1, IN]
       out  = x @ (num * s)^T + bias                (scale folded per-IN column)

Sharding (4x2 grid): 4-way split of the 8192 x-rows, 2-way split of OUT.

Per-core plan (all bulk DMA is contiguous HWDGE on the nc.sync ring; no
DMA-transpose, no SWDGE cast round-trips):

  phase 1: stream W [128 x IC] f32 tiles once.  For each 128x128 sub-tile,
           PE-transpose it into PSUM and accumulate the rank-16 dora^T
           (lhsT = ALPHA*A chunk, rhs = B^T chunk) into the same PSUM
           region, so PSUM holds num^T = W^T + ALPHA*(BA)^T directly.
           One DVE copy per [128, 512] PSUM block materializes a resident
           SBUF num^T (bf16, 16.8 MB).  Column norms come from a fused
           multiply-reduce over the finished num^T tiles (partition-major
           [128, IN/128] layout - no strided DRAM gathers).
  s:       16 KB partial colsum AllReduced across all 8 cores (each o-half
           contributed by MG cores -> reduce = MG * full; folded into the
           sqrt scale), then s = m / sqrt(.).
  phase 2: stream x [128 x IN] f32 row-tiles; PE-transpose each 128x128
           sub-tile; the mandatory PSUM->SBUF copy applies the per-i scale
           s (tensor_scalar_mul) and casts to bf16.  Dense bf16 GEMM with
           xT sub-tiles stationary (reused across 4 N=512 chunks), num^T
           moving, fp32 PSUM accumulation; bias (pre-replicated via a K=1
           ones-matmul) added on the PSUM drain; contiguous row stores.
"""

import sys

if "/opt/trn_rl_repo" not in sys.path:
    sys.path.insert(0, "/opt/trn_rl_repo")

import numpy as np

import concourse.bass as bass
import concourse.mybir as mybir
import concourse.tile as tile
from concourse import bacc
from concourse.bass_utils import run_bass_kernel_spmd
from concourse.masks import make_identity
from concourse.tile_rust import add_dep_helper

F32 = mybir.dt.float32
BF16 = mybir.dt.bfloat16

ALPHA = 16.0
N_CORES = 8
MG, OG = 4, 2  # core grid: 4 m-groups x 2 o-halves

# full problem sizes (hardcoded per contest contract)
B_, S_, IN_FULL, OUT_FULL, R_ = 4, 2048, 4096, 4096, 16
M_FULL = B_ * S_
M_C = M_FULL // MG      # 2048 x-rows per core
O_C = OUT_FULL // OG    # 2048 out-cols per core

FUSE_TDORA = True  # accumulate dora^T into the W^T transpose PSUM group


def build_kernel(M_C=M_C, IN=IN_FULL, O_C=O_C, R=R_, reps=1, loop_reps=1,
                 skip_cc=False, n_cores=N_CORES):
    """Build the (core-agnostic SPMD) bass program."""
    IC = min(1024, IN)        # phase-1 W-tile width (i)
    OB = min(512, O_C)        # phase-1 psum block width (o)
    NQ = min(512, O_C)        # phase-2 matmul free width (o)
    n_it = IN // 128
    assert IN % IC == 0 and O_C % OB == 0 and O_C % NQ == 0
    assert M_C % 128 == 0 and IC % 128 == 0 and OB % 128 == 0

    nc = bacc.Bacc("TRN2", target_bir_lowering=False, debug=False,
                   num_devices=n_cores)

    x_in = nc.dram_tensor("x_slice", [M_C, IN], F32, kind="ExternalInput")
    w_own = nc.dram_tensor("w_own", [O_C, IN], F32, kind="ExternalInput")
    bias_in = nc.dram_tensor("bias_own", [1, O_C], F32, kind="ExternalInput")
    m_in = nc.dram_tensor("m_row", [1, IN], F32, kind="ExternalInput")
    a_in = nc.dram_tensor("dora_a", [R, IN], F32, kind="ExternalInput")
    b_own = nc.dram_tensor("dora_b_own", [O_C, R], F32, kind="ExternalInput")
    out_t = nc.dram_tensor("out_slice", [M_C, O_C], F32, kind="ExternalOutput")

    s_drams, cc_outs = [], []
    for rep in range(reps):
        s_drams.append(nc.dram_tensor(f"s_dram{rep}", [128, n_it], F32))
        cc_outs.append(nc.dram_tensor(f"cc_out{rep}", [128, n_it], F32,
                                      addr_space="Shared"))

    with tile.TileContext(nc) as tc:
        if loop_reps > 1:
            with tc.For_i(0, loop_reps):
                _emit_rep(tc, nc, 0, M_C, IN, O_C, R, IC, OB, NQ, n_it,
                          x_in, w_own, bias_in, m_in, a_in, b_own, out_t,
                          s_drams[0], cc_outs[0], skip_cc)
        else:
            for rep in range(reps):
                _emit_rep(tc, nc, rep, M_C, IN, O_C, R, IC, OB, NQ, n_it,
                          x_in, w_own, bias_in, m_in, a_in, b_own, out_t,
                          s_drams[rep], cc_outs[rep], skip_cc)
    nc.compile()
    return nc


def _emit_rep(tc, nc, rep, M_C, IN, O_C, R, IC, OB, NQ, n_it,
              x_in, w_own, bias_in, m_in, a_in, b_own, out_t, s_dram, cc_out,
              skip_cc=False):
    ADD, MUL = mybir.AluOpType.add, mybir.AluOpType.mult
    n_ic = IN // IC
    n_ot = O_C // 128
    n_og = O_C // OB          # phase-1 o-groups (OB-wide)
    ob_t = OB // 128          # W row-tiles per o-group
    n_q = O_C // NQ

    with tc.tile_pool(name=f"c{rep}", bufs=1) as const, \
         tc.tile_pool(name=f"nt{rep}", bufs=1) as ntp:
        ident = const.tile([128, 128], F32, tag="ident")
        make_identity(nc, ident[:])
        bias_rep = const.tile([128, O_C], F32, tag="bias_rep")
        s_t = const.tile([128, n_it], F32, tag="s_t")
        nt_tiles = [ntp.tile([128, O_C], BF16, tag=f"nt{it}", name=f"nt{it}")
                    for it in range(n_it)]

        with tc.tile_pool(name=f"su{rep}", bufs=1) as setup, \
             tc.tile_pool(name=f"sups{rep}", bufs=2, space="PSUM") as sups:
            # ---- setup: A (pre-scaled), B^T, bias replicate, m ----
            a_bf = setup.tile([R, IN], BF16, tag="a_bf")
            bt_bf = setup.tile([R, O_C], BF16, tag="bt_bf")
            with tc.tile_pool(name=f"tmp{rep}", bufs=1) as tmp, \
                 tc.tile_pool(name=f"bt{rep}", bufs=2) as btp:
                ones_row = tmp.tile([1, 128], F32, tag="ones_row")
                nc.gpsimd.memset(ones_row[:], 1.0)

                a_raw = tmp.tile([R, IN], F32, tag="a_raw")
                nc.sync.dma_start(out=a_raw[:], in_=a_in[:, :])
                nc.vector.tensor_scalar_mul(a_bf[:], a_raw[:], ALPHA)

                for ot in range(n_ot):
                    b_t = btp.tile([128, R], F32, tag="b_t")
                    nc.sync.dma_start(out=b_t[:],
                                      in_=b_own[ot * 128:(ot + 1) * 128, :])
                    ps = sups.tile([R, 128], F32, tag="bt_ps")
                    nc.tensor.transpose(ps[:], b_t[:], ident[:])
                    nc.vector.tensor_copy(
                        out=bt_bf[:, ot * 128:(ot + 1) * 128], in_=ps[:])

                bias_sb = tmp.tile([1, O_C], F32, tag="bias_sb")
                nc.sync.dma_start(out=bias_sb[0:1, :], in_=bias_in[:, :])
                for oc in range(O_C // 512):
                    ps_b = sups.tile([128, 512], F32, tag="ps_b")
                    nc.tensor.matmul(ps_b[:], lhsT=ones_row[:],
                                     rhs=bias_sb[0:1, oc * 512:(oc + 1) * 512],
                                     start=True, stop=True)
                    nc.vector.tensor_copy(
                        out=bias_rep[:, oc * 512:(oc + 1) * 512], in_=ps_b[:])

            m_t = setup.tile([128, n_it], F32, tag="m_t")
            nc.sync.dma_start(
                out=m_t[:], in_=m_in.ap().rearrange("a (c p) -> (a p) c", p=128))

            # ---- phase 1: build resident num^T; fused colsum ----
            s_col = setup.tile([128, n_it], F32, tag="s_col")
            prod = setup.tile([128, O_C], BF16, tag="prod")
            with tc.tile_pool(name=f"w{rep}", bufs=ob_t + 2) as wp, \
                 tc.tile_pool(name=f"dt{rep}", bufs=3) as dtp, \
                 tc.tile_pool(name=f"p1ps{rep}", bufs=4, space="PSUM") as p1ps:
                for ic in range(n_ic):
                    for og in range(n_og):
                        w_ts = []
                        for j in range(ob_t):
                            ot = og * ob_t + j
                            w_t = wp.tile([128, IC], F32, tag="w_t")
                            nc.sync.dma_start(
                                out=w_t[:],
                                in_=w_own[ot * 128:(ot + 1) * 128,
                                          ic * IC:(ic + 1) * IC])
                            w_ts.append(w_t)
                        for s8 in range(IC // 128):
                            it = ic * (IC // 128) + s8
                            ps = p1ps.tile([128, OB], F32, tag="p1")
                            if FUSE_TDORA:
                                # dora^T first (start zeroes the whole bank),
                                # then W^T sub-transposes accumulate into it
                                nc.tensor.matmul(
                                    ps[:, :],
                                    lhsT=a_bf[:, it * 128:(it + 1) * 128],
                                    rhs=bt_bf[:, og * OB:(og + 1) * OB],
                                    start=True, stop=False)
                                for j in range(ob_t):
                                    nc.tensor.matmul(
                                        ps[:, j * 128:(j + 1) * 128],
                                        lhsT=w_ts[j][:, s8 * 128:(s8 + 1) * 128],
                                        rhs=ident[:], is_transpose=True,
                                        start=False, stop=(j == ob_t - 1))
                                nc.vector.tensor_copy(
                                    out=nt_tiles[it][:, og * OB:(og + 1) * OB],
                                    in_=ps[:])
                            else:
                                # TensorTensor may read only ONE input from
                                # PSUM: round-trip dora^T through SBUF.
                                for j in range(ob_t):
                                    nc.tensor.matmul(
                                        ps[:, j * 128:(j + 1) * 128],
                                        lhsT=w_ts[j][:, s8 * 128:(s8 + 1) * 128],
                                        rhs=ident[:], is_transpose=True,
                                        start=True, stop=True)
                                ps_d = p1ps.tile([128, OB], F32, tag="p1")
                                nc.tensor.matmul(
                                    ps_d[:, :],
                                    lhsT=a_bf[:, it * 128:(it + 1) * 128],
                                    rhs=bt_bf[:, og * OB:(og + 1) * OB],
                                    start=True, stop=True)
                                dt_sb = dtp.tile([128, OB], BF16, tag="dt_sb")
                                nc.vector.tensor_copy(out=dt_sb[:], in_=ps_d[:])
                                nc.vector.tensor_add(
                                    out=nt_tiles[it][:, og * OB:(og + 1) * OB],
                                    in0=ps[:], in1=dt_sb[:])
                    # colsum(num^2) partial for this ic's finished nt tiles:
                    # ACT square with accumulate (the softmax sum pattern)
                    for s8 in range(IC // 128):
                        it = ic * (IC // 128) + s8
                        nc.scalar.activation(
                            prod[:], nt_tiles[it][:],
                            mybir.ActivationFunctionType.Square,
                            0.0, 1.0, 0.0,
                            accum_out=s_col[:, it:it + 1])

            # ---- s = m / sqrt(allreduce(colsum) / MG) ----
            # store on gpsimd so the SWDGE store and the collective share a
            # queue (baseline-proven ordering; no explicit dep onto cc)
            st = nc.gpsimd.dma_start(out=s_dram[:, :], in_=s_col[:])
            s_raw = setup.tile([128, n_it], F32, tag="s_raw")
            if skip_cc:
                # timing-only build: collectives can't run inside For_i;
                # read back the local partial instead (same DMA shape)
                ld = nc.sync.dma_start(out=s_raw[:], in_=s_dram[:, :])
                add_dep_helper(ld.ins, st.ins, reason="s_raw RAW on s_dram")
            else:
                cc = nc.gpsimd.collective_compute(
                    "AllReduce", ADD,
                    ins=[s_dram.ap()], outs=[cc_out.ap()],
                    replica_groups=[list(range(N_CORES))])
                ld = nc.sync.dma_start(out=s_raw[:], in_=cc_out[:, :])
                add_dep_helper(ld.ins, cc.ins,
                               reason="s_raw RAW on collective out")
            s_sq = setup.tile([128, n_it], F32, tag="s_sq")
            nc.scalar.activation(s_sq[:], s_raw[:],
                                 mybir.ActivationFunctionType.Sqrt,
                                 0.0, 1.0 / MG)
            s_rc = setup.tile([128, n_it], F32, tag="s_rc")
            nc.vector.reciprocal(s_rc[:], s_sq[:])
            nc.vector.tensor_mul(out=s_t[:], in0=s_rc[:], in1=m_t[:])

        # ---- phase 2: out = (x^T * s)^T @ num^T + bias ----
        XH = min(2048, IN)  # x half-row width
        n_xh = IN // XH
        with tc.tile_pool(name=f"x{rep}", bufs=3) as xp, \
             tc.tile_pool(name=f"xs{rep}", bufs=n_it + 2) as xsp, \
             tc.tile_pool(name=f"ob{rep}", bufs=2) as obp, \
             tc.tile_pool(name=f"p2t{rep}", bufs=3, space="PSUM") as p2t, \
             tc.tile_pool(name=f"p2o{rep}", bufs=n_q + 1, space="PSUM") as p2o:
            for mt in range(M_C // 128):
                xs_tiles = []
                for xh in range(n_xh):
                    x_t = xp.tile([128, XH], F32, tag="x_t")
                    nc.sync.dma_start(
                        out=x_t[:],
                        in_=x_in[mt * 128:(mt + 1) * 128,
                                 xh * XH:(xh + 1) * XH])
                    for s8 in range(XH // 128):
                        it = xh * (XH // 128) + s8
                        ps_x = p2t.tile([128, 128], F32, tag="ps_x")
                        nc.tensor.transpose(
                            ps_x[:], x_t[:, s8 * 128:(s8 + 1) * 128], ident[:])
                        xs = xsp.tile([128, 128], BF16, tag="xs")
                        nc.vector.tensor_scalar_mul(xs[:], ps_x[:],
                                                    s_t[:, it:it + 1])
                        xs_tiles.append(xs)
                ps_q = [p2o.tile([128, NQ], F32, tag="ps_q", name="ps_q")
                        for _ in range(n_q)]
                for it in range(n_it):
                    for q in range(n_q):
                        nc.tensor.matmul(
                            ps_q[q][:, :],
                            lhsT=xs_tiles[it][:],
                            rhs=nt_tiles[it][:, q * NQ:(q + 1) * NQ],
                            start=(it == 0), stop=(it == n_it - 1))
                o_sb = obp.tile([128, O_C], F32, tag="o_sb")
                for q in range(n_q):
                    nc.vector.tensor_add(
                        out=o_sb[:, q * NQ:(q + 1) * NQ], in0=ps_q[q][:],
                        in1=bias_rep[:, q * NQ:(q + 1) * NQ])
                nc.sync.dma_start(
                    out=out_t[mt * 128:(mt + 1) * 128, :], in_=o_sb[:])


_NC_CACHE = {}


def get_nc(reps=1, loop_reps=1):
    key = (reps, loop_reps)
    if key not in _NC_CACHE:
        _NC_CACHE[key] = build_kernel(reps=reps, loop_reps=loop_reps)
    return _NC_CACHE[key]


def make_in_maps(x, weight, bias, m, dora_A, dora_B):
    x = np.ascontiguousarray(np.asarray(x, dtype=np.float32))
    weight = np.ascontiguousarray(np.asarray(weight, dtype=np.float32))
    bias = np.ascontiguousarray(np.asarray(bias, dtype=np.float32))
    m = np.ascontiguousarray(np.asarray(m, dtype=np.float32))
    dora_A = np.ascontiguousarray(np.asarray(dora_A, dtype=np.float32))
    dora_B = np.ascontiguousarray(np.asarray(dora_B, dtype=np.float32))
    xf = x.reshape(M_FULL, IN_FULL)
    in_maps = []
    for c in range(N_CORES):
        g, h = divmod(c, OG)
        o0 = h * O_C
        im = {
            "x_slice": np.ascontiguousarray(xf[g * M_C:(g + 1) * M_C]),
            "w_own": np.ascontiguousarray(weight[o0:o0 + O_C]),
            "bias_own": np.ascontiguousarray(bias[o0:o0 + O_C].reshape(1, O_C)),
            "m_row": np.ascontiguousarray(m.reshape(1, IN_FULL)),
            "dora_a": dora_A,
            "dora_b_own": np.ascontiguousarray(dora_B[o0:o0 + O_C]),
        }
        in_maps.append(im)
    return in_maps


def kernel(x, weight, bias, m, dora_A, dora_B, _trace=False, _trace_kwargs=None):
    in_maps = make_in_maps(x, weight, bias, m, dora_A, dora_B)
    res = run_bass_kernel_spmd(
        get_nc(), in_maps, core_ids=list(range(N_CORES)),
        trace=_trace, **(_trace_kwargs or {}))
    out = np.empty((M_FULL, OUT_FULL), np.float32)
    for c in range(N_CORES):
        g, h = divmod(c, OG)
        out[g * M_C:(g + 1) * M_C, h * O_C:(h + 1) * O_C] = \
            res.results[c]["out_slice"]
    ret = out.reshape(B_, S_, OUT_FULL)
    if _trace:
        return ret, res
    return ret
